# revision 2
# baseline (speedup 1.0000x reference)
"""Trainium2 Bass kernel for ApertureChamberSSM.

Computation (reference):
    iv, ov, beta_s, alpha, mg = sigmoid(scalars); decay = exp(-alpha)
    x_in  = iv * x
    drive = tanh(x_in)
    psi_s = decay * psi_{s-1} + (1-decay) * drive_s          (scan over S)
    x_mem = mg * psi + (1-mg) * x_in
    rotation of channel pairs (j, j+512) by angle pi*beta_s, scaled by ov

Algebra used here:  psi = (1-decay) * psi'  with  psi'_s = decay*psi'_{s-1} + drive_s
    x_mem = ap_*psi' + c*x      where ap_ = mg*(1-decay), c = (1-mg)*iv
    m'    = psi' + (c/ap_)*x    =>  x_mem = ap_ * m'
    out_r = cr*m'_r - ci*m'_i,  out_i = ci*m'_r + cr*m'_i
    with  cr = cos(pi*beta_s)*ov*ap_,  ci = sin(pi*beta_s)*ov*ap_

Sharding: 8 cores; core c owns channel pairs j in [64c, 64c+64) for all 4
batches -> 256 real rows + 256 imag rows of length S=8192 per core.
Row layout per core (512, 8192): rows [0:128]=R0, [128:256]=R1 (real,
(b*64+j) order), [256:384]=I0, [384:512]=I1; R-block partition p pairs with
I-block partition p.

On-core pipeline per [128, C] chunk: DMA in -> tanh on ACT ->
tensor_tensor_scan on DVE -> scalar_tensor_tensor blend on DVE (bf16 out) ->
pair rotation as scaled-identity bf16 matmuls on PE -> PSUM evict on ACT ->
DMA out.
"""

import math

import numpy as np

B, S, D = 4, 8192, 1024
HALF = D // 2          # 512
NCORES = 8
JPC = HALF // NCORES   # 64 channel pairs per core
ROWS = 2 * B * JPC     # 512 rows per core
P = 128                # partitions
C = 1024               # seq chunk (free dim) per tile
NCHUNK = S // C
NPAIR = ROWS // (2 * P)  # 2 block pairs: (R0,I0), (R1,I1)
MMF = 512              # matmul moving free dim (one PSUM bank)

_cache = {}


def _sig(v):
    return 1.0 / (1.0 + math.exp(-float(v)))


def _build(iv, decay, cb, cr, ci, use_scan):
    """Build + compile the 8-core SPMD Bacc graph with the given constants
    baked in. Returns the compiled Bacc."""
    import concourse.bass as bass
    import concourse.tile as tile
    from concourse import bacc, mybir

    f32 = mybir.dt.float32
    bf16 = mybir.dt.bfloat16
    AF = mybir.ActivationFunctionType
    OP = mybir.AluOpType

    nc = bacc.Bacc("TRN2", target_bir_lowering=False, debug=False,
                   num_devices=NCORES)
    x_ap = nc.dram_tensor("x", [ROWS, S], f32, kind="ExternalInput").ap()
    consts_ap = nc.dram_tensor("consts", [P, 3 * P], bf16,
                               kind="ExternalInput").ap()
    out_ap = nc.dram_tensor("out", [ROWS, S], f32, kind="ExternalOutput").ap()

    with tile.TileContext(nc) as tc:
        with (
            tc.tile_pool(name="const", bufs=1) as cpool,
            tc.tile_pool(name="xin", bufs=3) as xpool,
            tc.tile_pool(name="drv", bufs=2) as dpool,
            tc.tile_pool(name="psi", bufs=3) as ppool,
            tc.tile_pool(name="mm", bufs=2) as mpool,
            tc.tile_pool(name="outs", bufs=3) as opool,
            tc.tile_pool(name="ps", bufs=4, space=bass.MemorySpace.PSUM) as pspool,
        ):
            idm = cpool.tile([P, 3 * P], bf16, tag="idm")
            nc.sync.dma_start(idm[:], consts_ap[:])
            cr_t = idm[:, 0:P]        # cr * I
            ci_t = idm[:, P:2 * P]    # ci * I
            nci_t = idm[:, 2 * P:3 * P]  # -ci * I
            if use_scan:
                dk = cpool.tile([P, C], f32, tag="dk")
                nc.vector.memset(dk[:], decay)

            for i in range(NPAIR):
                prev = [None, None]  # carry APs for R and I chains
                for k in range(NCHUNK):
                    mt = []
                    for h in range(2):  # 0 = real block, 1 = imag block
                        r0 = h * 2 * P + i * P
                        xt = xpool.tile([P, C], f32, tag=f"x{h}")
                        nc.sync.dma_start(
                            xt[:], x_ap[r0:r0 + P, k * C:(k + 1) * C])
                        if use_scan:
                            dt = dpool.tile([P, C], f32, tag=f"d{h}")
                            nc.scalar.activation(dt[:], xt[:], AF.Tanh,
                                                 bias=0.0, scale=iv)
                            pt = ppool.tile([P, C], f32, tag=f"p{h}")
                            init = prev[h] if prev[h] is not None else 0.0
                            nc.vector.tensor_tensor_scan(
                                pt[:], dk[:], dt[:], init, OP.mult, OP.add)
                            prev[h] = pt[:, C - 1:C]
                            m = mpool.tile([P, C], bf16, tag=f"m{h}")
                            nc.vector.scalar_tensor_tensor(
                                m[:], xt[:], cb, pt[:], OP.mult, OP.add)
                        else:
                            # degenerate path (ap_ == 0): m' = x
                            m = mpool.tile([P, C], bf16, tag=f"m{h}")
                            nc.vector.tensor_copy(m[:], xt[:])
                        mt.append(m)

                    for h in range(2):
                        r0 = h * 2 * P + i * P
                        ot = opool.tile([P, C], f32, tag=f"o{h}")
                        for s4 in range(C // MMF):
                            fs = slice(s4 * MMF, (s4 + 1) * MMF)
                            ps = pspool.tile([P, MMF], f32, tag=f"ps{h}")
                            if h == 0:   # out_r = cr*m_r - ci*m_i
                                nc.tensor.matmul(ps[:], cr_t, mt[0][:, fs],
                                                 start=True, stop=False)
                                nc.tensor.matmul(ps[:], nci_t, mt[1][:, fs],
                                                 start=False, stop=True)
                            else:        # out_i = ci*m_r + cr*m_i
                                nc.tensor.matmul(ps[:], ci_t, mt[0][:, fs],
                                                 start=True, stop=False)
                                nc.tensor.matmul(ps[:], cr_t, mt[1][:, fs],
                                                 start=False, stop=True)
                            nc.scalar.copy(ot[:, fs], ps[:])
                        nc.sync.dma_start(
                            out_ap[r0:r0 + P, k * C:(k + 1) * C], ot[:])

    nc.compile()
    return nc


def kernel(x, beta, input_valve, output_valve, alpha_raw, memory_gate):
    x = np.asarray(x, dtype=np.float32)
    assert x.shape == (B, S, D), x.shape

    beta_s = _sig(beta)
    iv = _sig(input_valve)
    ov = _sig(output_valve)
    alpha = _sig(alpha_raw)
    mg = _sig(memory_gate)
    decay = math.exp(-alpha)
    c = (1.0 - mg) * iv
    ap_ = mg * (1.0 - decay)
    angle = math.pi * beta_s
    p_, q_ = math.cos(angle) * ov, math.sin(angle) * ov

    use_scan = abs(ap_) > 1e-12 * max(1.0, abs(c))
    if use_scan:
        cb = c / ap_
        cr, ci = p_ * ap_, q_ * ap_
    else:
        cb = 0.0
        cr, ci = p_ * c, q_ * c

    key = (round(iv, 12), round(decay, 12), round(cb, 12),
           round(cr, 12), round(ci, 12), use_scan)
    if key not in _cache:
        _cache[key] = _build(iv, decay, cb, cr, ci, use_scan)
    nc = _cache[key]

    import ml_dtypes
    from concourse.bass_utils import run_bass_kernel_spmd

    eye = np.eye(P, dtype=np.float64)
    consts = np.concatenate(
        [cr * eye, ci * eye, -ci * eye], axis=1).astype(ml_dtypes.bfloat16)

    # shard: core c gets channel pairs [64c, 64c+64) x 4 batches
    xr = x[:, :, :HALF].reshape(B, S, NCORES, JPC)
    xi = x[:, :, HALF:].reshape(B, S, NCORES, JPC)
    in_maps = []
    for cix in range(NCORES):
        shard = np.empty((ROWS, S), dtype=np.float32)
        shard[:ROWS // 2] = np.ascontiguousarray(
            xr[:, :, cix, :].transpose(0, 2, 1)).reshape(ROWS // 2, S)
        shard[ROWS // 2:] = np.ascontiguousarray(
            xi[:, :, cix, :].transpose(0, 2, 1)).reshape(ROWS // 2, S)
        in_maps.append({"x": shard, "consts": consts})

    res = run_bass_kernel_spmd(nc, in_maps, core_ids=list(range(NCORES)))
    global last_result
    last_result = res

    out = np.empty((B, S, D), dtype=np.float32)
    o_r = out[:, :, :HALF].reshape(B, S, NCORES, JPC)
    o_i = out[:, :, HALF:].reshape(B, S, NCORES, JPC)
    for cix in range(NCORES):
        oc = res.results[cix]["out"]
        o_r[:, :, cix, :] = oc[:ROWS // 2].reshape(
            B, JPC, S).transpose(0, 2, 1)
        o_i[:, :, cix, :] = oc[ROWS // 2:].reshape(
            B, JPC, S).transpose(0, 2, 1)
    return out


# revision 3
# speedup vs baseline: 1.2396x; 1.2396x over previous
"""Trainium2 Bass kernel for ApertureChamberSSM (v2).

Computation (reference):
    iv, ov, beta_s, alpha, mg = sigmoid(scalars); decay = exp(-alpha)
    x_in  = iv * x ; drive = tanh(x_in)
    psi_s = decay * psi_{s-1} + (1-decay) * drive_s          (scan over S)
    x_mem = mg * psi + (1-mg) * x_in
    rotate channel pairs (j, j+512) by pi*sigmoid(beta), scale by ov

Algebra: psi = (1-decay)*psi' with psi'_s = decay*psi'_{s-1} + drive_s
    x_mem = ap_*psi' + c*x   (ap_ = mg*(1-decay), c = (1-mg)*iv)
    out_r = p*x_mem_r - q*x_mem_i ; out_i = q*x_mem_r + p*x_mem_i
          (p = cos(pi*beta_s)*ov, q = sin(pi*beta_s)*ov)
    => out_r = (p*ap_)psi_r + (p*c)x_r + (-q*ap_)psi_i + (-q*c)x_i
       out_i = (q*ap_)psi_r + (q*c)x_r + ( p*ap_)psi_i + ( p*c)x_i
    i.e. a 4-term combination done as accumulated scaled-identity matmuls
    on the TensorEngine (channels pairs live at the same partition index of
    the R-block and I-block tiles).

Engine assignment per core: DMA bf16 in/out; tanh on ACT (f32 drive);
tensor_tensor_scan on DVE (bf16 psi out); blend+rotation fused on PE as
4 accumulated bf16 matmuls per PSUM tile; PSUM->SBUF eviction on ACT
(bf16); host does sigmoid/cos/sin and the (de)sharding/transposes.

Sharding: core c owns channel pairs j in [64c, 64c+64) for all 4 batches:
shard (512, 8192): rows [0:256] real (b*64+j order), [256:512] imag.
"""

import math

import numpy as np

B, S, D = 4, 8192, 1024
HALF = D // 2          # 512
NCORES = 8
JPC = HALF // NCORES   # 64 channel pairs per core
ROWS = 2 * B * JPC     # 512 rows per core
P = 128                # partitions
C = 2048               # seq chunk (free dim) per tile
NCHUNK = S // C
NPAIR = ROWS // (2 * P)  # 2 block pairs: (R0,I0), (R1,I1)
MMF = 512              # matmul moving free dim (one PSUM bank)

_cache = {}


def _sig(v):
    return 1.0 / (1.0 + math.exp(-float(v)))


def _build(iv, decay, use_scan):
    """Build + compile the 8-core SPMD graph. Rotation/blend coefficients
    arrive at runtime via the 'consts' input (8 scaled identities), so only
    iv, decay and the use_scan flag are baked in."""
    import concourse.bass as bass
    import concourse.tile as tile
    from concourse import bacc, mybir

    f32 = mybir.dt.float32
    bf16 = mybir.dt.bfloat16
    AF = mybir.ActivationFunctionType
    OP = mybir.AluOpType

    nc = bacc.Bacc("TRN2", target_bir_lowering=False, debug=False,
                   num_devices=NCORES)
    x_ap = nc.dram_tensor("x", [ROWS, S], bf16, kind="ExternalInput").ap()
    consts_ap = nc.dram_tensor("consts", [P, 8 * P], bf16,
                               kind="ExternalInput").ap()
    out_ap = nc.dram_tensor("out", [ROWS, S], bf16, kind="ExternalOutput").ap()

    with tile.TileContext(nc) as tc:
        with (
            tc.tile_pool(name="const", bufs=1) as cpool,
            tc.tile_pool(name="xin", bufs=3) as xpool,
            tc.tile_pool(name="drv", bufs=2) as dpool,
            tc.tile_pool(name="psi", bufs=3) as ppool,
            tc.tile_pool(name="outs", bufs=3) as opool,
            tc.tile_pool(name="ps", bufs=3, space=bass.MemorySpace.PSUM) as pspool,
        ):
            idm = cpool.tile([P, 8 * P], bf16, tag="idm")
            nc.sync.dma_start(idm[:], consts_ap[:])
            # identity blocks: [p*ap_, p*c, -q*ap_, -q*c, q*ap_, q*c, pad, pad]
            lhs = [idm[:, j * P:(j + 1) * P] for j in range(8)]
            # coefficient order per psum group: [psi_r, x_r, psi_i, x_i]
            coef_r = [lhs[0], lhs[1], lhs[2], lhs[3]]
            coef_i = [lhs[4], lhs[5], lhs[0], lhs[1]]

            if use_scan:
                dk = cpool.tile([P, C], f32, tag="dk")
                nc.vector.memset(dk[:], decay)

            for i in range(NPAIR):
                prev = [None, None]
                for k in range(NCHUNK):
                    xt, pt = [], []
                    for h in range(2):  # 0 = real block, 1 = imag block
                        r0 = h * 2 * P + i * P
                        x_t = xpool.tile([P, C], bf16, tag=f"x{h}")
                        nc.sync.dma_start(
                            x_t[:], x_ap[r0:r0 + P, k * C:(k + 1) * C])
                        xt.append(x_t)
                        if use_scan:
                            d_t = dpool.tile([P, C], f32, tag=f"d{h}")
                            nc.scalar.activation(d_t[:], x_t[:], AF.Tanh,
                                                 bias=0.0, scale=iv)
                            p_t = ppool.tile([P, C], bf16, tag=f"p{h}")
                            init = prev[h] if prev[h] is not None else 0.0
                            nc.vector.tensor_tensor_scan(
                                p_t[:], dk[:], d_t[:], init, OP.mult, OP.add)
                            prev[h] = p_t[:, C - 1:C]
                            pt.append(p_t)

                    for h in range(2):
                        r0 = h * 2 * P + i * P
                        coef = coef_r if h == 0 else coef_i
                        o_t = opool.tile([P, C], bf16, tag=f"o{h}")
                        for s4 in range(C // MMF):
                            fs = slice(s4 * MMF, (s4 + 1) * MMF)
                            ps = pspool.tile([P, MMF], f32, tag=f"ps{h}")
                            if use_scan:
                                srcs = [pt[0], xt[0], pt[1], xt[1]]
                            else:
                                srcs = [xt[0], xt[1]]
                                coef = [coef[1], coef[3]]
                            n = len(srcs)
                            for t in range(n):
                                nc.tensor.matmul(
                                    ps[:], coef[t], srcs[t][:, fs],
                                    start=(t == 0), stop=(t == n - 1))
                            nc.scalar.copy(o_t[:, fs], ps[:])
                        nc.sync.dma_start(
                            out_ap[r0:r0 + P, k * C:(k + 1) * C], o_t[:])

    nc.compile()
    return nc


def kernel(x, beta, input_valve, output_valve, alpha_raw, memory_gate):
    x = np.asarray(x, dtype=np.float32)
    assert x.shape == (B, S, D), x.shape

    beta_s = _sig(beta)
    iv = _sig(input_valve)
    ov = _sig(output_valve)
    alpha = _sig(alpha_raw)
    mg = _sig(memory_gate)
    decay = math.exp(-alpha)
    c = (1.0 - mg) * iv
    ap_ = mg * (1.0 - decay)
    angle = math.pi * beta_s
    p_, q_ = math.cos(angle) * ov, math.sin(angle) * ov
    use_scan = ap_ != 0.0

    key = (round(iv, 12), round(decay, 12), use_scan)
    if key not in _cache:
        _cache[key] = _build(iv, decay, use_scan)
    nc = _cache[key]

    import ml_dtypes
    from concourse.bass_utils import run_bass_kernel_spmd

    bf = ml_dtypes.bfloat16
    eye = np.eye(P, dtype=np.float64)
    blocks = [p_ * ap_, p_ * c, -q_ * ap_, -q_ * c, q_ * ap_, q_ * c, 0.0, 0.0]
    consts = np.concatenate([b * eye for b in blocks], axis=1).astype(bf)

    xr = x[:, :, :HALF].reshape(B, S, NCORES, JPC)
    xi = x[:, :, HALF:].reshape(B, S, NCORES, JPC)
    in_maps = []
    for cix in range(NCORES):
        shard = np.empty((ROWS, S), dtype=bf)
        shard[:ROWS // 2] = xr[:, :, cix, :].transpose(0, 2, 1).reshape(
            ROWS // 2, S).astype(bf)
        shard[ROWS // 2:] = xi[:, :, cix, :].transpose(0, 2, 1).reshape(
            ROWS // 2, S).astype(bf)
        in_maps.append({"x": shard, "consts": consts})

    res = run_bass_kernel_spmd(nc, in_maps, core_ids=list(range(NCORES)))
    global last_result
    last_result = res

    out = np.empty((B, S, D), dtype=np.float32)
    o_r = out[:, :, :HALF].reshape(B, S, NCORES, JPC)
    o_i = out[:, :, HALF:].reshape(B, S, NCORES, JPC)
    for cix in range(NCORES):
        oc = np.asarray(res.results[cix]["out"]).astype(np.float32)
        o_r[:, :, cix, :] = oc[:ROWS // 2].reshape(
            B, JPC, S).transpose(0, 2, 1)
        o_i[:, :, cix, :] = oc[ROWS // 2:].reshape(
            B, JPC, S).transpose(0, 2, 1)
    return out


# revision 6
# speedup vs baseline: 1.3656x; 1.1017x over previous
"""Trainium2 Bass kernel for ApertureChamberSSM (v2).

Computation (reference):
    iv, ov, beta_s, alpha, mg = sigmoid(scalars); decay = exp(-alpha)
    x_in  = iv * x ; drive = tanh(x_in)
    psi_s = decay * psi_{s-1} + (1-decay) * drive_s          (scan over S)
    x_mem = mg * psi + (1-mg) * x_in
    rotate channel pairs (j, j+512) by pi*sigmoid(beta), scale by ov

Algebra: psi = (1-decay)*psi' with psi'_s = decay*psi'_{s-1} + drive_s
    x_mem = ap_*psi' + c*x   (ap_ = mg*(1-decay), c = (1-mg)*iv)
    out_r = p*x_mem_r - q*x_mem_i ; out_i = q*x_mem_r + p*x_mem_i
          (p = cos(pi*beta_s)*ov, q = sin(pi*beta_s)*ov)
    => out_r = (p*ap_)psi_r + (p*c)x_r + (-q*ap_)psi_i + (-q*c)x_i
       out_i = (q*ap_)psi_r + (q*c)x_r + ( p*ap_)psi_i + ( p*c)x_i
    i.e. a 4-term combination done as accumulated scaled-identity matmuls
    on the TensorEngine (channels pairs live at the same partition index of
    the R-block and I-block tiles).

Engine assignment per core: DMA bf16 in/out; tanh on ACT (f32 drive);
tensor_tensor_scan on DVE (bf16 psi out); blend+rotation fused on PE as
4 accumulated bf16 matmuls per PSUM tile; PSUM->SBUF eviction on ACT
(bf16); host does sigmoid/cos/sin and the (de)sharding/transposes.

Sharding: core c owns channel pairs j in [64c, 64c+64) for all 4 batches:
shard (512, 8192): rows [0:256] real (b*64+j order), [256:512] imag.
"""

import math

import numpy as np

B, S, D = 4, 8192, 1024
HALF = D // 2          # 512
NCORES = 8
JPC = HALF // NCORES   # 64 channel pairs per core
ROWS = 2 * B * JPC     # 512 rows per core
P = 128                # partitions
C = 1024               # seq chunk (free dim) per tile
NCHUNK = S // C
NPAIR = ROWS // (2 * P)  # 2 block pairs: (R0,I0), (R1,I1)
MMF = 512              # matmul moving free dim (one PSUM bank)
SCG = 512              # scan chaining granularity

_cache = {}


def _sig(v):
    return 1.0 / (1.0 + math.exp(-float(v)))


def _build(iv, decay, use_scan):
    """Build + compile the 8-core SPMD graph. Rotation/blend coefficients
    arrive at runtime via the 'consts' input (8 scaled identities), so only
    iv, decay and the use_scan flag are baked in."""
    import concourse.bass as bass
    import concourse.tile as tile
    from concourse import bacc, mybir

    f32 = mybir.dt.float32
    bf16 = mybir.dt.bfloat16
    AF = mybir.ActivationFunctionType
    OP = mybir.AluOpType

    nc = bacc.Bacc("TRN2", target_bir_lowering=False, debug=False,
                   num_devices=NCORES)
    x_ap = nc.dram_tensor("x", [ROWS, S], bf16, kind="ExternalInput").ap()
    consts_ap = nc.dram_tensor("consts", [P, 8 * P], bf16,
                               kind="ExternalInput").ap()
    out_ap = nc.dram_tensor("out", [ROWS, S], bf16, kind="ExternalOutput").ap()

    with tile.TileContext(nc) as tc:
        with (
            tc.tile_pool(name="const", bufs=1) as cpool,
            tc.tile_pool(name="xin", bufs=4) as xpool,
            tc.tile_pool(name="drv", bufs=3) as dpool,
            tc.tile_pool(name="psi", bufs=4) as ppool,
            tc.tile_pool(name="outs", bufs=4) as opool,
            tc.tile_pool(name="ps", bufs=1, space=bass.MemorySpace.PSUM) as pspool,
        ):
            idm = cpool.tile([P, 8 * P], bf16, tag="idm")
            nc.sync.dma_start(idm[:], consts_ap[:])
            # identity blocks: [p*ap_, p*c, -q*ap_, -q*c, q*ap_, q*c, pad, pad]
            lhs = [idm[:, j * P:(j + 1) * P] for j in range(8)]
            # coefficient order per psum group: [psi_r, x_r, psi_i, x_i]
            coef_r = [lhs[0], lhs[1], lhs[2], lhs[3]]
            coef_i = [lhs[4], lhs[5], lhs[0], lhs[1]]

            if use_scan:
                dk = cpool.tile([P, C], f32, tag="dk")
                nc.vector.memset(dk[:], decay)

            prev = [[None, None] for _ in range(NPAIR)]
            for k in range(NCHUNK):
                for i in range(NPAIR):
                    xt, pt = [], []
                    for h in range(2):  # 0 = real block, 1 = imag block
                        r0 = h * 2 * P + i * P
                        x_t = xpool.tile([P, C], bf16, tag=f"x{i}{h}")
                        nc.sync.dma_start(
                            x_t[:], x_ap[r0:r0 + P, k * C:(k + 1) * C])
                        xt.append(x_t)
                        if use_scan:
                            d_t = dpool.tile([P, C], f32, tag=f"d{i}{h}")
                            nc.scalar.activation(d_t[:], x_t[:], AF.Tanh,
                                                 bias=0.0, scale=iv)
                            p_t = ppool.tile([P, C], bf16, tag=f"p{i}{h}")
                            for g in range(C // SCG):
                                gs = slice(g * SCG, (g + 1) * SCG)
                                init = (prev[i][h] if prev[i][h] is not None
                                        else 0.0)
                                nc.vector.tensor_tensor_scan(
                                    p_t[:, gs], dk[:, gs], d_t[:, gs], init,
                                    OP.mult, OP.add)
                                prev[i][h] = p_t[:, (g + 1) * SCG - 1:
                                                 (g + 1) * SCG]
                            pt.append(p_t)

                    for h in range(2):
                        r0 = h * 2 * P + i * P
                        coef = coef_r if h == 0 else coef_i
                        if use_scan:
                            srcs = [pt[0], xt[0], pt[1], xt[1]]
                        else:
                            srcs = [xt[0], xt[1]]
                            coef = [coef[1], coef[3]]
                        o_t = opool.tile([P, C], bf16, tag=f"o{i}{h}")
                        ps = pspool.tile([P, C], f32, tag=f"ps{i}{h}")
                        nsub = C // MMF
                        for t in range(len(srcs)):  # term-outer: 1 ldw/term
                            for s4 in range(nsub):
                                fs = slice(s4 * MMF, (s4 + 1) * MMF)
                                nc.tensor.matmul(
                                    ps[:, fs], coef[t], srcs[t][:, fs],
                                    start=(t == 0), stop=(t == len(srcs) - 1))
                        nc.scalar.copy(o_t[:], ps[:])
                        nc.sync.dma_start(
                            out_ap[r0:r0 + P, k * C:(k + 1) * C], o_t[:])

    nc.compile()
    return nc


def kernel(x, beta, input_valve, output_valve, alpha_raw, memory_gate):
    x = np.asarray(x, dtype=np.float32)
    assert x.shape == (B, S, D), x.shape

    beta_s = _sig(beta)
    iv = _sig(input_valve)
    ov = _sig(output_valve)
    alpha = _sig(alpha_raw)
    mg = _sig(memory_gate)
    decay = math.exp(-alpha)
    c = (1.0 - mg) * iv
    ap_ = mg * (1.0 - decay)
    angle = math.pi * beta_s
    p_, q_ = math.cos(angle) * ov, math.sin(angle) * ov
    use_scan = ap_ != 0.0

    key = (round(iv, 12), round(decay, 12), use_scan)
    if key not in _cache:
        _cache[key] = _build(iv, decay, use_scan)
    nc = _cache[key]

    import ml_dtypes
    from concourse.bass_utils import run_bass_kernel_spmd

    bf = ml_dtypes.bfloat16
    eye = np.eye(P, dtype=np.float64)
    blocks = [p_ * ap_, p_ * c, -q_ * ap_, -q_ * c, q_ * ap_, q_ * c, 0.0, 0.0]
    consts = np.concatenate([b * eye for b in blocks], axis=1).astype(bf)

    xr = x[:, :, :HALF].reshape(B, S, NCORES, JPC)
    xi = x[:, :, HALF:].reshape(B, S, NCORES, JPC)
    in_maps = []
    for cix in range(NCORES):
        shard = np.empty((ROWS, S), dtype=bf)
        shard[:ROWS // 2] = xr[:, :, cix, :].transpose(0, 2, 1).reshape(
            ROWS // 2, S).astype(bf)
        shard[ROWS // 2:] = xi[:, :, cix, :].transpose(0, 2, 1).reshape(
            ROWS // 2, S).astype(bf)
        in_maps.append({"x": shard, "consts": consts})

    res = run_bass_kernel_spmd(nc, in_maps, core_ids=list(range(NCORES)))
    global last_result
    last_result = res

    out = np.empty((B, S, D), dtype=np.float32)
    o_r = out[:, :, :HALF].reshape(B, S, NCORES, JPC)
    o_i = out[:, :, HALF:].reshape(B, S, NCORES, JPC)
    for cix in range(NCORES):
        oc = np.asarray(res.results[cix]["out"]).astype(np.float32)
        o_r[:, :, cix, :] = oc[:ROWS // 2].reshape(
            B, JPC, S).transpose(0, 2, 1)
        o_i[:, :, cix, :] = oc[ROWS // 2:].reshape(
            B, JPC, S).transpose(0, 2, 1)
    return out


# revision 7
# speedup vs baseline: 1.3788x; 1.0097x over previous
"""Trainium2 Bass kernel for ApertureChamberSSM (v2).

Computation (reference):
    iv, ov, beta_s, alpha, mg = sigmoid(scalars); decay = exp(-alpha)
    x_in  = iv * x ; drive = tanh(x_in)
    psi_s = decay * psi_{s-1} + (1-decay) * drive_s          (scan over S)
    x_mem = mg * psi + (1-mg) * x_in
    rotate channel pairs (j, j+512) by pi*sigmoid(beta), scale by ov

Algebra: psi = (1-decay)*psi' with psi'_s = decay*psi'_{s-1} + drive_s
    x_mem = ap_*psi' + c*x   (ap_ = mg*(1-decay), c = (1-mg)*iv)
    out_r = p*x_mem_r - q*x_mem_i ; out_i = q*x_mem_r + p*x_mem_i
          (p = cos(pi*beta_s)*ov, q = sin(pi*beta_s)*ov)
    => out_r = (p*ap_)psi_r + (p*c)x_r + (-q*ap_)psi_i + (-q*c)x_i
       out_i = (q*ap_)psi_r + (q*c)x_r + ( p*ap_)psi_i + ( p*c)x_i
    i.e. a 4-term combination done as accumulated scaled-identity matmuls
    on the TensorEngine (channels pairs live at the same partition index of
    the R-block and I-block tiles).

Engine assignment per core: DMA bf16 in/out; tanh on ACT (f32 drive);
tensor_tensor_scan on DVE (bf16 psi out); blend+rotation fused on PE as
4 accumulated bf16 matmuls per PSUM tile; PSUM->SBUF eviction on ACT
(bf16); host does sigmoid/cos/sin and the (de)sharding/transposes.

Sharding: core c owns channel pairs j in [64c, 64c+64) for all 4 batches:
shard (512, 8192): rows [0:256] real (b*64+j order), [256:512] imag.
"""

import math

import numpy as np

B, S, D = 4, 8192, 1024
HALF = D // 2          # 512
NCORES = 8
JPC = HALF // NCORES   # 64 channel pairs per core
ROWS = 2 * B * JPC     # 512 rows per core
P = 128                # partitions
C = 1024               # seq chunk (free dim) per tile
NCHUNK = S // C
NPAIR = ROWS // (2 * P)  # 2 block pairs: (R0,I0), (R1,I1)
MMF = 512              # matmul moving free dim (one PSUM bank)
SCG = 1024             # scan chaining granularity

_cache = {}


def _sig(v):
    return 1.0 / (1.0 + math.exp(-float(v)))


def _build(iv, decay, use_scan):
    """Build + compile the 8-core SPMD graph. Rotation/blend coefficients
    arrive at runtime via the 'consts' input (8 scaled identities), so only
    iv, decay and the use_scan flag are baked in."""
    import concourse.bass as bass
    import concourse.tile as tile
    from concourse import bacc, mybir

    f32 = mybir.dt.float32
    bf16 = mybir.dt.bfloat16
    AF = mybir.ActivationFunctionType
    OP = mybir.AluOpType

    nc = bacc.Bacc("TRN2", target_bir_lowering=False, debug=False,
                   num_devices=NCORES)
    x_ap = nc.dram_tensor("x", [ROWS, S], bf16, kind="ExternalInput").ap()
    consts_ap = nc.dram_tensor("consts", [P, 8 * P], bf16,
                               kind="ExternalInput").ap()
    out_ap = nc.dram_tensor("out", [ROWS, S], bf16, kind="ExternalOutput").ap()

    with tile.TileContext(nc) as tc:
        with (
            tc.tile_pool(name="const", bufs=1) as cpool,
            tc.tile_pool(name="xin", bufs=4) as xpool,
            tc.tile_pool(name="drv", bufs=3) as dpool,
            tc.tile_pool(name="psi", bufs=4) as ppool,
            tc.tile_pool(name="outs", bufs=4) as opool,
            tc.tile_pool(name="ps", bufs=1, space=bass.MemorySpace.PSUM) as pspool,
        ):
            idm = cpool.tile([P, 8 * P], bf16, tag="idm")
            nc.sync.dma_start(idm[:], consts_ap[:])
            # identity blocks: [p*ap_, p*c, -q*ap_, -q*c, q*ap_, q*c, pad, pad]
            lhs = [idm[:, j * P:(j + 1) * P] for j in range(8)]
            # coefficient order per psum group: [psi_r, x_r, psi_i, x_i]
            coef_r = [lhs[0], lhs[1], lhs[2], lhs[3]]
            coef_i = [lhs[4], lhs[5], lhs[0], lhs[1]]

            if use_scan:
                dk = cpool.tile([P, C], f32, tag="dk")
                nc.vector.memset(dk[:], decay)

            prev = [[None, None] for _ in range(NPAIR)]
            for k in range(NCHUNK):
                for i in range(NPAIR):
                    xt, pt = [], []
                    for h in range(2):  # 0 = real block, 1 = imag block
                        r0 = h * 2 * P + i * P
                        x_t = xpool.tile([P, C], bf16, tag=f"x{i}{h}")
                        nc.sync.dma_start(
                            x_t[:], x_ap[r0:r0 + P, k * C:(k + 1) * C])
                        xt.append(x_t)
                        if use_scan:
                            d_t = dpool.tile([P, C], f32, tag=f"d{i}{h}")
                            nc.scalar.activation(d_t[:], x_t[:], AF.Tanh,
                                                 bias=0.0, scale=iv)
                            p_t = ppool.tile([P, C], bf16, tag=f"p{i}{h}")
                            for g in range(C // SCG):
                                gs = slice(g * SCG, (g + 1) * SCG)
                                init = (prev[i][h] if prev[i][h] is not None
                                        else 0.0)
                                nc.vector.tensor_tensor_scan(
                                    p_t[:, gs], dk[:, gs], d_t[:, gs], init,
                                    OP.mult, OP.add)
                                prev[i][h] = p_t[:, (g + 1) * SCG - 1:
                                                 (g + 1) * SCG]
                            pt.append(p_t)

                    for h in range(2):
                        r0 = h * 2 * P + i * P
                        coef = coef_r if h == 0 else coef_i
                        if use_scan:
                            srcs = [pt[0], xt[0], pt[1], xt[1]]
                        else:
                            srcs = [xt[0], xt[1]]
                            coef = [coef[1], coef[3]]
                        o_t = opool.tile([P, C], bf16, tag=f"o{i}{h}")
                        ps = pspool.tile([P, C], f32, tag=f"ps{i}{h}")
                        nsub = C // MMF
                        for t in range(len(srcs)):  # term-outer: 1 ldw/term
                            for s4 in range(nsub):
                                fs = slice(s4 * MMF, (s4 + 1) * MMF)
                                nc.tensor.matmul(
                                    ps[:, fs], coef[t], srcs[t][:, fs],
                                    start=(t == 0), stop=(t == len(srcs) - 1))
                        nc.scalar.copy(o_t[:], ps[:])
                        nc.sync.dma_start(
                            out_ap[r0:r0 + P, k * C:(k + 1) * C], o_t[:])

    nc.compile()
    return nc


def kernel(x, beta, input_valve, output_valve, alpha_raw, memory_gate):
    x = np.asarray(x, dtype=np.float32)
    assert x.shape == (B, S, D), x.shape

    beta_s = _sig(beta)
    iv = _sig(input_valve)
    ov = _sig(output_valve)
    alpha = _sig(alpha_raw)
    mg = _sig(memory_gate)
    decay = math.exp(-alpha)
    c = (1.0 - mg) * iv
    ap_ = mg * (1.0 - decay)
    angle = math.pi * beta_s
    p_, q_ = math.cos(angle) * ov, math.sin(angle) * ov
    use_scan = ap_ != 0.0

    key = (round(iv, 12), round(decay, 12), use_scan)
    if key not in _cache:
        _cache[key] = _build(iv, decay, use_scan)
    nc = _cache[key]

    import ml_dtypes
    from concourse.bass_utils import run_bass_kernel_spmd

    bf = ml_dtypes.bfloat16
    eye = np.eye(P, dtype=np.float64)
    blocks = [p_ * ap_, p_ * c, -q_ * ap_, -q_ * c, q_ * ap_, q_ * c, 0.0, 0.0]
    consts = np.concatenate([b * eye for b in blocks], axis=1).astype(bf)

    xr = x[:, :, :HALF].reshape(B, S, NCORES, JPC)
    xi = x[:, :, HALF:].reshape(B, S, NCORES, JPC)
    in_maps = []
    for cix in range(NCORES):
        shard = np.empty((ROWS, S), dtype=bf)
        shard[:ROWS // 2] = xr[:, :, cix, :].transpose(0, 2, 1).reshape(
            ROWS // 2, S).astype(bf)
        shard[ROWS // 2:] = xi[:, :, cix, :].transpose(0, 2, 1).reshape(
            ROWS // 2, S).astype(bf)
        in_maps.append({"x": shard, "consts": consts})

    res = run_bass_kernel_spmd(nc, in_maps, core_ids=list(range(NCORES)))
    global last_result
    last_result = res

    out = np.empty((B, S, D), dtype=np.float32)
    o_r = out[:, :, :HALF].reshape(B, S, NCORES, JPC)
    o_i = out[:, :, HALF:].reshape(B, S, NCORES, JPC)
    for cix in range(NCORES):
        oc = np.asarray(res.results[cix]["out"]).astype(np.float32)
        o_r[:, :, cix, :] = oc[:ROWS // 2].reshape(
            B, JPC, S).transpose(0, 2, 1)
        o_i[:, :, cix, :] = oc[ROWS // 2:].reshape(
            B, JPC, S).transpose(0, 2, 1)
    return out
